# revision 19
# baseline (speedup 1.0000x reference)
"""Trainium2 Bass kernel for the nGPT-style dense transformer block (v4).

Data-parallel: one batch element per NeuronCore.  v3 changes (kept):
  * k computed directly feature-major (k^T = Wkn @ h^T, fp8 DoubleRow);
    per-head ||k||^2 via an indicator matmul over the partition dim,
    transposed [16,T]->[T,16] on the PE into the exp scale.
  * residual algebra uses justnorm's scale invariance:
      justnorm((1-lr) h^ + lr b^) = justnorm(h + b * s),
      s = lr/(1-lr) * ||h||/||b||   (one Sqrt with a folded per-token
    input-scale AP + one reciprocal).
  * htm / h2 / k-square staging in bf16; vz/qz zero-fills hoisted out of
    the loop (zero lanes are never overwritten).
v4: software pipelining.  The bench loop body computes attention(i) and
the MLP phases of i while EMITTING the QKV/norm stages of i+1 interleaved
into the MLP sections (whose PSUM budget has room), with a P(0) prologue
before the hardware loop.  Engines see independent work between the
dependency chains of each phase:
    body(i): A(i) | [O(i) ~ K(i+1)] | esc | [T2(i) ~ Q(i+1)] |
             F1(i) | [F2(i) ~ V(i+1) ~ hn(i+1)]
kfm/esc/qz/vz/htm/rs2i persist across trips (written for i+1 in trip i).
"""

import numpy as np
import ml_dtypes

import concourse.bass as bass
import concourse.mybir as mybir
import concourse.tile as tile
from concourse import bacc
BF16 = ml_dtypes.bfloat16
FP8 = ml_dtypes.float8_e4m3
F32 = mybir.dt.float32
BF = mybir.dt.bfloat16
F8 = mybir.dt.float8e4
MM8 = mybir.MatmulPerfMode.DoubleRow
AF = mybir.ActivationFunctionType
AX = mybir.AxisListType
ALU = mybir.AluOpType

P = 128
T = 1024
C = 1024
H = 16
D = 64
F = 8192
NCORES = 8
TCH = T // P   # 8 token chunks
CCH = C // P   # 8 channel chunks
KCH = (F // 2) // P  # 32 chunks of the 4096-dim MLP mid

BASE_SCALE = 0.03125
ATTN_ALPHA_INIT = 0.05
MLP_ALPHA_INIT = 0.05
SQK_INIT = 1.0
SUV_INIT = 1.0

WSCALE = 16.0     # host scale on normalized W columns (cancels exactly)
EXP_BIAS = -3.0   # exp(logit + bias); positive row scale cancels in justnorm
YSCALE = 1.0 / 16.0  # y -> fp8 eviction scale (cancels in justnorm)

_COMPILED: dict = {}


class _Pools:
    def __init__(self, tc):
        self.tc = tc
        self._open = {}

    def open(self, name, **kw):
        cm = self.tc.tile_pool(name=name, **kw)
        pool = cm.__enter__()
        self._open[name] = cm
        return pool

    def close(self, *names):
        for name in names:
            cm = self._open.pop(name)
            cm.__exit__(None, None, None)

    def close_all(self):
        for name in reversed(list(self._open)):
            self.close(name)


def _declare_io(nc):
    io = {}
    io["htf"] = nc.dram_tensor("htf", [P, CCH, T], F8, kind="ExternalInput")
    io["htm"] = nc.dram_tensor("htm", [TCH, P, C], BF, kind="ExternalInput")
    io["wq"] = nc.dram_tensor("wq", [P, CCH, C], F8, kind="ExternalInput")
    io["wk"] = nc.dram_tensor("wk", [P, CCH, C], F8, kind="ExternalInput")
    io["wv"] = nc.dram_tensor("wv", [P, CCH, C], F8, kind="ExternalInput")
    io["wo"] = nc.dram_tensor("wo", [P, CCH, C], F8, kind="ExternalInput")
    io["wfc"] = nc.dram_tensor("wfc", [16, P, CCH, 512], F8, kind="ExternalInput")
    io["wpj"] = nc.dram_tensor("wpj", [P, KCH, C], F8, kind="ExternalInput")
    io["esc8"] = nc.dram_tensor("esc8", [P, H], F32, kind="ExternalInput")
    io["ind16"] = nc.dram_tensor("ind16", [P, CCH, H], BF, kind="ExternalInput")
    io["ident"] = nc.dram_tensor("ident", [P, P], BF, kind="ExternalInput")
    io["out"] = nc.dram_tensor("out", [TCH, P, C], F32, kind="ExternalOutput")
    return io


def _emit_preamble(nc, pl, io):
    """Constants + all state that crosses hardware-loop trip boundaries."""
    consts = pl.open("consts", bufs=1)
    st = {}
    st["ident"] = consts.tile([P, P], BF, name="ident")
    nc.sync.dma_start(out=st["ident"], in_=io["ident"].ap())
    st["esc8"] = consts.tile([P, H], F32, name="esc8")
    nc.sync.dma_start(out=st["esc8"], in_=io["esc8"].ap())
    st["ind16"] = consts.tile([P, CCH, H], BF, name="ind16")
    nc.sync.dma_start(out=st["ind16"], in_=io["ind16"].ap())
    st["ebias"] = consts.tile([P, 1], F32, name="ebias")
    nc.vector.memset(st["ebias"], EXP_BIAS)

    persist = pl.open("persist", bufs=1)
    # v in fp8, two half-zeroed copies: vz[s] has head-parity s features
    # live and the other parity zero, so AV DoubleRow can use M=128
    # stationaries that write both sub-heads' PSUM rows in one chain.
    st["vz"] = [persist.tile([P, TCH, C], F8, name=f"vz{s}") for s in range(2)]
    # feature-major q-hat, zero-padded per head (the other sub-head's 64
    # rows stay zero) so score matmuls run dense K=128 stationaries.
    st["qz"] = persist.tile([P, H, T], F8, name="qz")
    st["kfm"] = persist.tile([P, CCH, T], F8, name="kfm")
    st["esc_all"] = persist.tile([P, TCH, H], F32, name="esc_all")
    st["htm"] = persist.tile([P, TCH, C], BF, name="htm_sb")
    st["rs2i"] = persist.tile([P, TCH], F32, name="rs2i")
    for s in range(2):
        nc.vector.memset(st["vz"][s], 0)
    nc.vector.memset(st["qz"], 0)
    return st


def _weave(*unit_lists):
    """Emit thunks from several lists interleaved proportionally."""
    lists = [list(u) for u in unit_lists if u]
    idx = [0] * len(lists)
    total = sum(len(u) for u in lists)
    for _ in range(total):
        best = min(
            (i for i in range(len(lists)) if idx[i] < len(lists[i])),
            key=lambda i: idx[i] / len(lists[i]),
        )
        lists[best][idx[best]]()
        idx[best] += 1


class _Iter:
    """Emitters for one logical block iteration.  P-stage emitters write
    the persistent tiles (kfm/esc/qz/vz/htm/rs2i)."""

    def __init__(self, nc, io, st, pl, lr_a, lr_m):
        self.nc, self.io, self.st, self.pl = nc, io, st, pl
        self.lr_a, self.lr_m = lr_a, lr_m

    # ---------------- input DMAs ----------------
    def emit_input_dmas(self, htf, w_sbs, with_htm=True):
        nc, io, st = self.nc, self.io, self.st
        nc.sync.dma_start(out=w_sbs["wo"], in_=io["wo"].ap())
        nc.sync.dma_start(out=htf, in_=io["htf"].ap())
        for nm in ("wk", "wq", "wv"):
            nc.sync.dma_start(out=w_sbs[nm], in_=io[nm].ap())
        if with_htm:
            self.emit_htm_dma()

    def emit_htm_dma(self):
        nc, io, st = self.nc, self.io, self.st
        nc.sync.dma_start(out=st["htm"],
                          in_=io["htm"].ap().rearrange("t p c -> p t c"))

    # ------------ h row norms: rs2i = ((1-la)/la)^2 / ||h||^2 ------------
    def hn_units(self, hnscr):
        nc, st = self.nc, self.st
        la = self.lr_a / (1.0 - self.lr_a)

        def unit(it):
            nscr = hnscr.tile([P, C], BF, name="nscr", tag="nscr")
            nc.scalar.activation(nscr, st["htm"][:, it, :], AF.Square,
                                 accum_out=st["rs2i"][:, it:it + 1])

        def tail():
            nc.vector.reciprocal(st["rs2i"], st["rs2i"])
            nc.vector.tensor_scalar_mul(st["rs2i"], st["rs2i"],
                                        1.0 / (la * la))

        return [lambda it=it: unit(it) for it in range(TCH)] + [tail]

    # ------------ K stage: k^T direct + per-head norms ------------
    def k_units(self, htf, wk, kps, rkps, ksqp):
        nc, st = self.nc, self.st
        rkp = rkps.tile([16, 2, 512], F32, name="rkp", tag="rkp")

        def unit(ci):
            kt = kps.tile([P, T], F32, name="kt", tag="kt")
            for cp in range(CCH // 2):
                lhs = wk[:, 2 * cp:2 * cp + 2, ci * P:(ci + 1) * P]
                for hf in range(2):
                    nc.tensor.matmul(
                        kt[:, hf * 512:(hf + 1) * 512], lhs,
                        htf[:, 2 * cp:2 * cp + 2, hf * 512:(hf + 1) * 512],
                        perf_mode=MM8,
                        start=(cp == 0), stop=(cp == CCH // 2 - 1),
                    )
            nc.vector.tensor_copy(st["kfm"][:, ci, :], kt)
            ksq = ksqp.tile([P, T], BF, name="ksq", tag="ksq")
            nc.scalar.activation(ksq, kt, AF.Square)
            for hf in range(2):
                nc.tensor.matmul(
                    rkp[:, hf], st["ind16"][:, ci, :],
                    ksq[:, hf * 512:(hf + 1) * 512],
                    start=(ci == 0), stop=(ci == CCH - 1),
                )

        self._rkp = rkp
        return [lambda ci=ci: unit(ci) for ci in range(CCH)]

    def esc_tail(self, escps, small):
        nc, st = self.nc, self.st
        rk_sb = small.tile([16, T], BF, name="rk_sb", tag="rk_sb")
        nc.vector.tensor_copy(rk_sb, self._rkp.rearrange("p a b -> p (a b)"))
        for tk in range(TCH):
            esct = escps.tile([P, H], BF, name="esct", tag="esct")
            nc.tensor.transpose(esct, rk_sb[:, tk * P:(tk + 1) * P],
                                st["ident"][0:16, 0:16])
            em = small.tile([P, H], F32, name="em", tag="em")
            nc.vector.reciprocal(em, esct)
            nc.scalar.sqrt(em, em)
            nc.vector.tensor_mul(st["esc_all"][:, tk, :], em, st["esc8"])

    # ------------ Q stage: token-major + normalize + transpose ------------
    def q_units(self, htf, wq, qhat, qps, tqps, qscr, small):
        nc, st = self.nc, self.st

        def q_tp(ci, g):
            tq = tqps.tile([P, 4, P], BF, name="tq", tag="tq")
            for jj in range(4):
                it_ = g * 4 + jj
                nc.tensor.transpose(
                    tq[:, jj], qhat[:, it_, ci * P:(ci + 1) * P], st["ident"]
                )
            tqv = tq.rearrange("p a b -> p (a b)")
            for sub in range(2):
                h = 2 * ci + sub
                nc.vector.tensor_copy(
                    st["qz"][sub * D:(sub + 1) * D, h, g * 512:(g + 1) * 512],
                    tqv[sub * D:(sub + 1) * D, :],
                )

        def unit(it):
            psq = qps.tile([P, 2, 512], F32, name="psq", tag="psq")
            for cp in range(CCH // 2):
                lhs = htf[:, 2 * cp:2 * cp + 2, it * P:(it + 1) * P]
                for hf in range(2):
                    nc.tensor.matmul(
                        psq[:, hf], lhs,
                        wq[:, 2 * cp:2 * cp + 2, hf * 512:(hf + 1) * 512],
                        perf_mode=MM8,
                        start=(cp == 0), stop=(cp == CCH // 2 - 1),
                    )
            psqv = psq.rearrange("p a b -> p (a b)")
            sqt = qscr.tile([P, C], BF, name="sqt", tag="sqt")
            nc.scalar.activation(sqt, psqv, AF.Square)
            rq = small.tile([P, H], F32, name="rq", tag="rq")
            nc.vector.reduce_sum(rq, sqt.rearrange("p (h d) -> p h d", h=H),
                                 axis=AX.X)
            nc.vector.reciprocal(rq, rq)
            nc.scalar.sqrt(rq, rq)
            nc.vector.tensor_mul(
                qhat[:, it, :].rearrange("p (h d) -> p h d", h=H),
                psq.rearrange("p a (g d) -> p (a g) d", d=D),
                rq.to_broadcast((P, H, D)),
            )

        units = []
        for it in range(TCH):
            units.append(lambda it=it: unit(it))
            if it == 3:
                units.extend(lambda ci=ci: q_tp(ci, 0) for ci in range(CCH))
        units.extend(lambda ci=ci: q_tp(ci, 1) for ci in range(CCH))
        return units

    # ------------ V stage: token-major -> vz interleaved fp8 ------------
    def v_units(self, htf, wv, vps):
        nc, st = self.nc, self.st

        def unit(it):
            psv = vps.tile([P, 2, 512], F32, name="psv", tag="psv")
            for cp in range(CCH // 2):
                lhs = htf[:, 2 * cp:2 * cp + 2, it * P:(it + 1) * P]
                for hf in range(2):
                    nc.tensor.matmul(
                        psv[:, hf], lhs,
                        wv[:, 2 * cp:2 * cp + 2, hf * 512:(hf + 1) * 512],
                        perf_mode=MM8,
                        start=(cp == 0), stop=(cp == CCH // 2 - 1),
                    )
            psv_v = psv.rearrange("p a (i d) -> p a i d", d=D)
            for s in range(2):
                nc.vector.tensor_copy(
                    st["vz"][s][:, it, :]
                    .rearrange("p (a i d) -> p a i d", a=2, d=D)[:, :, s::2, :],
                    psv_v[:, :, s::2, :],
                )

        return [lambda it=it: unit(it) for it in range(TCH)]

    # ---------------- Phase A: attention ----------------
    def emit_A(self, yfm, aps, app):
        nc, st = self.nc, self.st
        for hp in range(H // 2):
            ypsum = aps.tile([P, 2, 512], F32, name="ypsum", tag="ypsum",
                             bufs=1)
            p_sb = [
                app.tile([P, TCH, T], F8, name=f"p{sub}", tag=f"p{sub}")
                for sub in range(2)
            ]
            for tk in range(TCH):
                sps = []
                for sub in range(2):
                    h = hp * 2 + sub
                    sp = aps.tile([P, 2, 512], F32, name="sp", tag="sp",
                                  bufs=2)
                    for hf in range(2):
                        nc.tensor.matmul(
                            sp[:, hf],
                            st["kfm"][:, hp, tk * P:(tk + 1) * P],
                            st["qz"][:, h, hf * 512:(hf + 1) * 512],
                            start=True, stop=True,
                        )
                    sps.append(sp)
                for sub in range(2):
                    h = hp * 2 + sub
                    nc.scalar.activation(
                        out=p_sb[sub][:, tk, :],
                        in_=sps[sub].rearrange("p a b -> p (a b)"),
                        func=AF.Exp,
                        scale=st["esc_all"][:, tk, h:h + 1],
                        bias=st["ebias"],
                    )
                if tk % 2 == 1:
                    m = tk // 2
                    for sub in range(2):
                        for hf in range(2):
                            nc.tensor.matmul(
                                ypsum[:, hf],
                                st["vz"][sub][:, 2 * m:2 * m + 2,
                                              hp * P:(hp + 1) * P],
                                p_sb[sub][:, 2 * m:2 * m + 2,
                                          hf * 512:(hf + 1) * 512],
                                perf_mode=MM8,
                                start=(m == 0 and sub == 0),
                                stop=(m == TCH // 2 - 1 and sub == 1),
                            )
            nc.vector.tensor_scalar_mul(
                yfm[:, hp, :], ypsum.rearrange("p a b -> p (a b)"), YSCALE
            )

    # ------------ Phase O: out-proj + attention residual ------------
    def o_units(self, yfm, wo, h2a, opsp, oscr, small):
        nc, st = self.nc, self.st

        def unit(it):
            ops = opsp.tile([P, 2, 512], F32, name="ops", tag="ops")
            for cp in range(CCH // 2):
                lhs = yfm[:, 2 * cp:2 * cp + 2, it * P:(it + 1) * P]
                for hf in range(2):
                    nc.tensor.matmul(
                        ops[:, hf], lhs,
                        wo[:, 2 * cp:2 * cp + 2, hf * 512:(hf + 1) * 512],
                        perf_mode=MM8,
                        start=(cp == 0), stop=(cp == CCH // 2 - 1),
                    )
            opsv = ops.rearrange("p a b -> p (a b)")
            sb = small.tile([P, 1], F32, name="sb", tag="sb")
            nsq = oscr.tile([P, C], BF, name="nsq", tag="nsq")
            nc.scalar.activation(nsq, opsv, AF.Square, accum_out=sb)
            nc.scalar.activation(sb, sb, AF.Sqrt,
                                 scale=st["rs2i"][:, it:it + 1])
            nc.vector.reciprocal(sb, sb)
            acc = oscr.tile([P, C], F32, name="acc", tag="acc")
            nc.vector.scalar_tensor_tensor(
                out=acc, in0=opsv, scalar=sb, in1=st["htm"][:, it, :],
                op0=ALU.mult, op1=ALU.add,
            )
            s2 = small.tile([P, 1], F32, name="s2", tag="s2")
            nsq2 = oscr.tile([P, C], BF, name="nsq2", tag="nsq2")
            nc.scalar.activation(nsq2, acc, AF.Square, accum_out=s2)
            nc.scalar.sqrt(s2, s2)
            nc.vector.reciprocal(s2, s2)
            nc.gpsimd.tensor_scalar_mul(h2a[:, it, :], acc, s2)

        return [lambda it=it: unit(it) for it in range(TCH)]

    # ------------ Phase T2: h2 -> feature-major fp8 ------------
    def t2_units(self, h2a, h2fm, tpps2):
        nc, st = self.nc, self.st

        def unit(ci, g):
            tp2 = tpps2.tile([P, 4, P], BF, name="tp2", tag="tp2")
            for jj in range(4):
                it = g * 4 + jj
                nc.tensor.transpose(
                    tp2[:, jj], h2a[:, it, ci * P:(ci + 1) * P], st["ident"]
                )
            nc.vector.tensor_scalar_mul(
                h2fm[:, ci, g * 512:(g + 1) * 512],
                tp2.rearrange("p a b -> p (a b)"), 8.0,
            )

        return [lambda ci=ci, g=g: unit(ci, g)
                for ci in range(CCH) for g in range(2)]

    # ------------ Phase F1: MLP up + SwiGLU ------------
    def emit_F1(self, h2fm, xm, f1w, f1ps, f1scr):
        nc, io = self.nc, self.io
        for j in range(8):
            wu = f1w.tile([P, CCH, 512], F8, name="wu", tag="wu")
            nc.sync.dma_start(out=wu, in_=io["wfc"].ap()[j])
            wvt = f1w.tile([P, CCH, 512], F8, name="wvt", tag="wvt")
            nc.sync.dma_start(out=wvt, in_=io["wfc"].ap()[j + 8])
            for so in range(4):
                oc = j * 4 + so
                m0 = so * P
                up = f1ps.tile([P, 2, 512], F32, name="up", tag="up")
                vp = f1ps.tile([P, 2, 512], F32, name="vp", tag="vp")
                for cp in range(CCH // 2):
                    for hf in range(2):
                        nc.tensor.matmul(
                            up[:, hf], wu[:, 2 * cp:2 * cp + 2, m0:m0 + P],
                            h2fm[:, 2 * cp:2 * cp + 2,
                                 hf * 512:(hf + 1) * 512],
                            perf_mode=MM8,
                            start=(cp == 0), stop=(cp == CCH // 2 - 1),
                        )
                for cp in range(CCH // 2):
                    for hf in range(2):
                        nc.tensor.matmul(
                            vp[:, hf], wvt[:, 2 * cp:2 * cp + 2, m0:m0 + P],
                            h2fm[:, 2 * cp:2 * cp + 2,
                                 hf * 512:(hf + 1) * 512],
                            perf_mode=MM8,
                            start=(cp == 0), stop=(cp == CCH // 2 - 1),
                        )
                sil = f1scr.tile([P, T], BF, name="sil", tag="sil")
                nc.scalar.activation(
                    out=sil, in_=vp.rearrange("p a b -> p (a b)"),
                    func=AF.Silu, scale=1.0 / 8.0,
                )
                nc.vector.tensor_mul(
                    xm[:, oc, :], up.rearrange("p a b -> p (a b)"), sil
                )

    # ------------ Phase F2: MLP down + MLP residual ------------
    def f2_units(self, xm, wpj, h2a, f2ps, f2scr, small):
        nc, io = self.nc, self.io
        lm = self.lr_m / (1.0 - self.lr_m)

        def unit(it):
            mp = f2ps.tile([P, 2, 512], F32, name="mp", tag="mp")
            for kp in range(KCH // 2):
                for hf in range(2):
                    nc.tensor.matmul(
                        mp[:, hf],
                        xm[:, 2 * kp:2 * kp + 2, it * P:(it + 1) * P],
                        wpj[:, 2 * kp:2 * kp + 2, hf * 512:(hf + 1) * 512],
                        perf_mode=MM8,
                        start=(kp == 0), stop=(kp == KCH // 2 - 1),
                    )
            mpv = mp.rearrange("p a b -> p (a b)")
            sb2 = small.tile([P, 1], F32, name="sb2", tag="sb2")
            nsq3 = f2scr.tile([P, C], BF, name="nsq3", tag="nsq3")
            nc.scalar.activation(nsq3, mpv, AF.Square, accum_out=sb2)
            nc.scalar.activation(sb2, sb2, AF.Sqrt, scale=1.0 / (lm * lm))
            nc.vector.reciprocal(sb2, sb2)
            acc2 = f2scr.tile([P, C], F32, name="acc2", tag="acc2")
            nc.vector.scalar_tensor_tensor(
                out=acc2, in0=mpv, scalar=sb2, in1=h2a[:, it, :],
                op0=ALU.mult, op1=ALU.add,
            )
            s3 = small.tile([P, 1], F32, name="s3", tag="s3")
            nsq4 = f2scr.tile([P, C], BF, name="nsq4", tag="nsq4")
            nc.scalar.activation(nsq4, acc2, AF.Square, accum_out=s3)
            nc.scalar.sqrt(s3, s3)
            nc.vector.reciprocal(s3, s3)
            outt = f2scr.tile([P, C], F32, name="outt", tag="outt")
            nc.scalar.mul(outt, acc2, s3)
            nc.scalar.dma_start(out=io["out"].ap()[it], in_=outt)

        return [lambda it=it: unit(it) for it in range(TCH)]


def _emit_P_flat(nc, em, pl, pfx=""):
    """QKV/norm stages, standalone order (prologue and flat path).
    Returns w_sbs (wo still needed by the caller's O phase)."""
    htfp = pl.open(pfx + "htfp", bufs=1)
    wqkp = pl.open(pfx + "wqkp", bufs=1)
    qhatp = pl.open(pfx + "qhatp", bufs=1)
    psmall = pl.open(pfx + "psmall", bufs=4)
    htf = htfp.tile([P, CCH, T], F8, name="htf_sb")
    w_sbs = {
        "wk": wqkp.tile([P, CCH, C], F8, name="wk_sb"),
        "wq": wqkp.tile([P, CCH, C], F8, name="wq_sb"),
        "wv": wqkp.tile([P, CCH, C], F8, name="wv_sb"),
        "wo": wqkp.tile([P, CCH, C], F8, name="wo_sb"),
    }
    em.emit_input_dmas(htf, w_sbs)
    hnscr = pl.open(pfx + "hnscr", bufs=2, side="right")
    for u in em.hn_units(hnscr):
        u()
    pl.close(pfx + "hnscr")
    rkps = pl.open(pfx + "rk_psum", bufs=1, space="PSUM")
    kps = pl.open(pfx + "k_psum", bufs=2, space="PSUM")
    ksqp = pl.open(pfx + "ksqp", bufs=3, side="right")
    for u in em.k_units(htf, w_sbs["wk"], kps, rkps, ksqp):
        u()
    pl.close(pfx + "k_psum")
    escps = pl.open(pfx + "esc_psum", bufs=2, space="PSUM")
    em.esc_tail(escps, psmall)
    pl.close(pfx + "esc_psum", pfx + "ksqp", pfx + "rk_psum")
    qhat = qhatp.tile([P, TCH, C], BF, name="qhat")
    qps = pl.open(pfx + "q_psum", bufs=2, space="PSUM")
    tqps = pl.open(pfx + "tq_psum", bufs=2, space="PSUM")
    qscr = pl.open(pfx + "qscr", bufs=2)
    for u in em.q_units(htf, w_sbs["wq"], qhat, qps, tqps, qscr, psmall):
        u()
    pl.close(pfx + "qscr", pfx + "tq_psum", pfx + "q_psum")
    vps = pl.open(pfx + "v_psum", bufs=2, space="PSUM")
    for u in em.v_units(htf, w_sbs["wv"], vps):
        u()
    pl.close(pfx + "v_psum")
    return w_sbs


def _emit_flat(nc, tc, io, st, pl, lr_a, lr_m, stop_after="full"):
    """One self-contained iteration: P, A, O, T2, F1, F2 (graded path)."""
    em = _Iter(nc, io, st, pl, lr_a, lr_m)

    def _dump_and_stop(srcs):
        dmp = pl.open("dmp", bufs=2)
        for i, s in enumerate(srcs[:TCH]):
            dt_ = dmp.tile([P, C], F32, name=f"dt{i}", tag="dt")
            nc.vector.tensor_copy(dt_, s)
            nc.sync.dma_start(out=io["out"].ap()[i], in_=dt_)
        pl.close("dmp")
        for name in [n for n in reversed(list(pl._open))
                     if n not in ("consts", "persist")]:
            pl.close(name)

    h2ap = pl.open("h2ap", bufs=1)
    h2a = h2ap.tile([P, TCH, C], BF, name="h2a")
    small = pl.open("small", bufs=4)
    w_sbs = _emit_P_flat(nc, em, pl)
    if stop_after == "p":
        _dump_and_stop([st["kfm"][:, i, :] for i in range(TCH)])
        return

    yfmp = pl.open("yfmp", bufs=1, side="right")
    yfm = yfmp.tile([P, CCH, T], F8, name="yfm")
    aps = pl.open("att_psum", bufs=1, space="PSUM")
    app = pl.open("att_p", bufs=2)
    em.emit_A(yfm, aps, app)
    pl.close("att_p", "att_psum")
    if stop_after == "a":
        _dump_and_stop([yfm[:, i, :] for i in range(TCH)])
        return

    opsp = pl.open("o_psum", bufs=3, space="PSUM")
    oscr = pl.open("o_scr", bufs=3)
    for u in em.o_units(yfm, w_sbs["wo"], h2a, opsp, oscr, small):
        u()
    pl.close("o_scr", "o_psum", "yfmp")
    if stop_after == "o":
        _dump_and_stop([h2a[:, i, :] for i in range(TCH)])
        return
    pl.close("psmall", "qhatp", "wqkp", "htfp")

    h2fmp = pl.open("h2fmp", bufs=1, side="right")
    h2fm = h2fmp.tile([P, CCH, T], F8, name="h2fm")
    tpps2 = pl.open("tp2_psum", bufs=3, space="PSUM")
    for u in em.t2_units(h2a, h2fm, tpps2):
        u()
    pl.close("tp2_psum")

    xmp = pl.open("xmp", bufs=1)
    xm = xmp.tile([P, KCH, T], F8, name="xm")
    wpjp = pl.open("wpjp", bufs=1)
    wpj = wpjp.tile([P, KCH, C], F8, name="wpj")
    for q in range(4):
        nc.sync.dma_start(out=wpj[:, q * 8:(q + 1) * 8, :],
                          in_=io["wpj"].ap()[:, q * 8:(q + 1) * 8, :])
    f1w = pl.open("f1w", bufs=3, side="right")
    f1ps = pl.open("f1_psum", bufs=2, space="PSUM")
    f1scr = pl.open("f1scr", bufs=2, side="right")
    em.emit_F1(h2fm, xm, f1w, f1ps, f1scr)
    pl.close("f1scr", "f1w", "h2fmp", "f1_psum")

    f2ps = pl.open("f2_psum", bufs=3, space="PSUM")
    f2scr = pl.open("f2scr", bufs=3)
    for u in em.f2_units(xm, wpj, h2a, f2ps, f2scr, small):
        u()
    pl.close("f2scr", "f2_psum", "wpjp", "xmp")
    pl.close("small", "h2ap")


def _emit_pipe_body(nc, tc, io, st, pl, lr_a, lr_m):
    """Pipelined loop body: A(i) + MLP(i), with P(i+1) stages woven into
    the MLP sections."""
    em = _Iter(nc, io, st, pl, lr_a, lr_m)

    htfp = pl.open("htfp", bufs=1)
    h2ap = pl.open("h2ap", bufs=1)
    small = pl.open("small", bufs=4)
    wvp = pl.open("wvp", bufs=1)
    wqp = pl.open("wqp", bufs=1)
    wkop = pl.open("wkop", bufs=1)
    htf = htfp.tile([P, CCH, T], F8, name="htf_sb")
    w_sbs = {
        "wv": wvp.tile([P, CCH, C], F8, name="wv_sb"),
        "wq": wqp.tile([P, CCH, C], F8, name="wq_sb"),
        "wk": wkop.tile([P, CCH, C], F8, name="wk_sb"),
        "wo": wkop.tile([P, CCH, C], F8, name="wo_sb"),
    }
    h2a = h2ap.tile([P, TCH, C], BF, name="h2a")
    # inputs for i+1 (WAR semaphores gate each against its last reader);
    # htm(i+1) must wait until after O(i)'s htm(i) reads, so it is
    # emitted at the T2~Q section instead.
    em.emit_input_dmas(htf, w_sbs, with_htm=False)

    # ---- A(i) ----
    yfmp = pl.open("yfmp", bufs=1, side="right")
    yfm = yfmp.tile([P, CCH, T], F8, name="yfm")
    aps = pl.open("att_psum", bufs=1, space="PSUM")
    app = pl.open("att_p", bufs=2)
    em.emit_A(yfm, aps, app)
    pl.close("att_p", "att_psum")

    # ---- O(i) ~ K(i+1) ----
    rkps = pl.open("rk_psum", bufs=1, space="PSUM")
    opsp = pl.open("o_psum", bufs=2, space="PSUM")
    oscr = pl.open("o_scr", bufs=3)
    kps = pl.open("k_psum", bufs=1, space="PSUM")
    ksqp = pl.open("ksqp", bufs=3, side="right")
    _weave(em.o_units(yfm, w_sbs["wo"], h2a, opsp, oscr, small),
           em.k_units(htf, w_sbs["wk"], kps, rkps, ksqp))
    pl.close("k_psum", "o_scr", "o_psum")
    escps = pl.open("esc_psum", bufs=2, space="PSUM")
    em.esc_tail(escps, small)
    pl.close("esc_psum", "ksqp", "rk_psum", "yfmp", "wkop")

    # ---- T2(i) ~ Q(i+1) ----
    em.emit_htm_dma()
    qhatp = pl.open("qhatp", bufs=1)
    qhat = qhatp.tile([P, TCH, C], BF, name="qhat")
    h2fmp = pl.open("h2fmp", bufs=1, side="right")
    h2fm = h2fmp.tile([P, CCH, T], F8, name="h2fm")
    tpps2 = pl.open("tp2_psum", bufs=2, space="PSUM")
    qps = pl.open("q_psum", bufs=2, space="PSUM")
    tqps = pl.open("tq_psum", bufs=2, space="PSUM")
    qscr = pl.open("qscr", bufs=2)
    _weave(em.t2_units(h2a, h2fm, tpps2),
           em.q_units(htf, w_sbs["wq"], qhat, qps, tqps, qscr, small))
    pl.close("qscr", "tq_psum", "q_psum", "tp2_psum", "qhatp", "wqp")

    # ---- F1(i) ----
    xmp = pl.open("xmp", bufs=1)
    xm = xmp.tile([P, KCH, T], F8, name="xm")
    wpjp = pl.open("wpjp", bufs=1)
    wpj = wpjp.tile([P, KCH, C], F8, name="wpj")
    for q in range(4):
        nc.sync.dma_start(out=wpj[:, q * 8:(q + 1) * 8, :],
                          in_=io["wpj"].ap()[:, q * 8:(q + 1) * 8, :])
    f1w = pl.open("f1w", bufs=3, side="right")
    f1ps = pl.open("f1_psum", bufs=2, space="PSUM")
    f1scr = pl.open("f1scr", bufs=2, side="right")
    em.emit_F1(h2fm, xm, f1w, f1ps, f1scr)
    pl.close("f1scr", "f1w", "h2fmp", "f1_psum")

    # ---- F2(i) ~ V(i+1) ~ hn(i+1) ----
    f2ps = pl.open("f2_psum", bufs=2, space="PSUM")
    f2scr = pl.open("f2scr", bufs=3)
    vps = pl.open("v_psum", bufs=2, space="PSUM")
    hnscr = pl.open("hnscr", bufs=2, side="right")
    _weave(em.f2_units(xm, wpj, h2a, f2ps, f2scr, small),
           em.v_units(htf, w_sbs["wv"], vps),
           em.hn_units(hnscr))
    pl.close("hnscr", "v_psum", "f2scr", "f2_psum", "wpjp", "xmp")
    pl.close("wvp", "small", "h2ap", "htfp")


def build_program(lr_a: float, lr_m: float, reps: int = 1, loop: int = 0,
                  stop_after: str = "full"):
    key = (round(lr_a, 12), round(lr_m, 12), reps, loop, stop_after)
    if key in _COMPILED:
        return _COMPILED[key]
    nc = bacc.Bacc("TRN2", target_bir_lowering=False, debug=False, num_devices=NCORES)
    with tile.TileContext(nc) as tc:
        io = _declare_io(nc)
        pl = _Pools(tc)
        st = _emit_preamble(nc, pl, io)
        if loop:
            em0 = _Iter(nc, io, st, pl, lr_a, lr_m)
            _emit_P_flat(nc, em0, pl, pfx="pro_")
            pl.close("pro_psmall", "pro_qhatp", "pro_wqkp", "pro_htfp")
            with tc.For_i(0, loop, 1):
                _emit_pipe_body(nc, tc, io, st, pl, lr_a, lr_m)
        else:
            for _ in range(reps):
                _emit_flat(nc, tc, io, st, pl, lr_a, lr_m, stop_after)
        pl.close_all()
    nc.compile()
    _COMPILED[key] = nc
    return nc


def prep_inputs(h, Wq, Wk, Wv, Wo, Wfc, Wproj, sqk, suv, attn_alpha, mlp_alpha):
    """Host-side sharding/layout. Returns (in_maps list per core, lr_a, lr_m)."""
    h = np.asarray(h, np.float32)
    Wq, Wk, Wv, Wo = (np.asarray(w, np.float64) for w in (Wq, Wk, Wv, Wo))
    Wfc = np.asarray(Wfc, np.float32)
    Wproj = np.asarray(Wproj, np.float32)
    sqk = np.asarray(sqk, np.float64)
    suv = np.asarray(suv, np.float64)
    attn_alpha = np.asarray(attn_alpha, np.float64)
    mlp_alpha = np.asarray(mlp_alpha, np.float64)

    sqk_s = sqk * (SQK_INIT / BASE_SCALE)
    s_h = sqk_s.reshape(H, D)
    assert np.allclose(s_h, s_h[:, :1]), "sqk must be constant per head"
    s2 = (s_h[:, 0] ** 2) * np.sqrt(D)
    esc8 = np.ascontiguousarray(np.broadcast_to(s2.astype(np.float32), (P, H)))

    lr_a_v = np.abs(attn_alpha * (ATTN_ALPHA_INIT / BASE_SCALE))
    lr_m_v = np.abs(mlp_alpha * (MLP_ALPHA_INIT / BASE_SCALE))
    assert np.allclose(lr_a_v, lr_a_v[0]) and np.allclose(lr_m_v, lr_m_v[0]), \
        "alpha must be constant"
    lr_a = float(lr_a_v[0])
    lr_m = float(lr_m_v[0])

    def wt_tiles_norm_f8(W):  # [out, in] -> [128, CCH, out] fp8, unit cols x16
        Wn = W / np.linalg.norm(W, axis=0, keepdims=True)
        Wn = Wn * WSCALE
        return np.ascontiguousarray(
            Wn.T.reshape(CCH, P, W.shape[0]).transpose(1, 0, 2)
        ).astype(FP8)

    wq_t, wk_t, wv_t, wo_t = (wt_tiles_norm_f8(w) for w in (Wq, Wk, Wv, Wo))

    suv_s = suv * (SUV_INIT / 1.0 * np.sqrt(C))
    wfc_f = (Wfc.astype(np.float64) * suv_s[:, None]).astype(np.float32)  # [F, C]
    wfc_t = np.ascontiguousarray(
        np.clip(wfc_f.T.reshape(CCH, P, 16, 512).transpose(2, 1, 0, 3), -224, 224)
    ).astype(FP8)
    wpj_t = np.ascontiguousarray(
        np.clip(Wproj.T.reshape(KCH, P, C).transpose(1, 0, 2) * 64.0, -224, 224)
    ).astype(FP8)

    # indicator stationaries for the per-head ||k||^2 partition reduction:
    # variant ci maps partitions [0,64) -> head 2ci, [64,128) -> head 2ci+1
    ind16 = np.zeros((P, CCH, H), dtype=BF16)
    for ci in range(CCH):
        ind16[0:D, ci, 2 * ci] = 1.0
        ind16[D:P, ci, 2 * ci + 1] = 1.0

    shared = {
        "wq": wq_t, "wk": wk_t, "wv": wv_t, "wo": wo_t,
        "wfc": wfc_t, "wpj": wpj_t, "esc8": esc8, "ind16": ind16,
        "ident": np.eye(P, dtype=np.float32).astype(BF16),
    }
    in_maps = []
    for b in range(NCORES):
        htf = np.ascontiguousarray(
            h[b].T.reshape(CCH, P, T).transpose(1, 0, 2)
        ).astype(FP8)
        htm = np.ascontiguousarray(h[b].reshape(TCH, P, C)).astype(BF16)
        in_maps.append({"htf": htf, "htm": htm, **shared})
    return in_maps, lr_a, lr_m


def kernel(h, Wq, Wk, Wv, Wo, Wfc, Wproj, sqk, suv, attn_alpha, mlp_alpha):
    in_maps, lr_a, lr_m = prep_inputs(
        h, Wq, Wk, Wv, Wo, Wfc, Wproj, sqk, suv, attn_alpha, mlp_alpha
    )
    nc = build_program(lr_a, lr_m)
    from concourse.bass_utils import run_bass_kernel_spmd

    res = run_bass_kernel_spmd(nc, in_maps, core_ids=list(range(NCORES)))
    out = np.stack(
        [res.results[b]["out"].reshape(T, C) for b in range(NCORES)], axis=0
    )
    return out.astype(np.float32)


# revision 23
# speedup vs baseline: 1.0816x; 1.0816x over previous
"""Trainium2 Bass kernel for the nGPT-style dense transformer block (v4).

Data-parallel: one batch element per NeuronCore.  v3 changes (kept):
  * k computed directly feature-major (k^T = Wkn @ h^T, fp8 DoubleRow);
    per-head ||k||^2 via an indicator matmul over the partition dim,
    transposed [16,T]->[T,16] on the PE into the exp scale.
  * residual algebra uses justnorm's scale invariance:
      justnorm((1-lr) h^ + lr b^) = justnorm(h + b * s),
      s = lr/(1-lr) * ||h||/||b||   (one Sqrt with a folded per-token
    input-scale AP + one reciprocal).
  * htm / h2 / k-square staging in bf16; vz/qz zero-fills hoisted out of
    the loop (zero lanes are never overwritten).
v4: software pipelining.  The bench loop body computes attention(i) and
the MLP phases of i while EMITTING the QKV/norm stages of i+1 interleaved
into the MLP sections (whose PSUM budget has room), with a P(0) prologue
before the hardware loop.  Engines see independent work between the
dependency chains of each phase:
    body(i): A(i) | [O(i) ~ K(i+1)] | esc | [T2(i) ~ Q(i+1)] |
             F1(i) | [F2(i) ~ V(i+1) ~ hn(i+1)]
kfm/esc/qz/vz/htm/rs2i persist across trips (written for i+1 in trip i).
"""

import numpy as np
import ml_dtypes

import concourse.bass as bass
import concourse.mybir as mybir
import concourse.tile as tile
from concourse import bacc
BF16 = ml_dtypes.bfloat16
FP8 = ml_dtypes.float8_e4m3
F32 = mybir.dt.float32
BF = mybir.dt.bfloat16
F8 = mybir.dt.float8e4
MM8 = mybir.MatmulPerfMode.DoubleRow
AF = mybir.ActivationFunctionType
AX = mybir.AxisListType
ALU = mybir.AluOpType

P = 128
T = 1024
C = 1024
H = 16
D = 64
F = 8192
NCORES = 8
TCH = T // P   # 8 token chunks
CCH = C // P   # 8 channel chunks
KCH = (F // 2) // P  # 32 chunks of the 4096-dim MLP mid

BASE_SCALE = 0.03125
ATTN_ALPHA_INIT = 0.05
MLP_ALPHA_INIT = 0.05
SQK_INIT = 1.0
SUV_INIT = 1.0

WSCALE = 16.0     # host scale on normalized W columns (cancels exactly)
EXP_BIAS = -3.0   # exp(logit + bias); positive row scale cancels in justnorm
YSCALE = 1.0 / 16.0  # y -> fp8 eviction scale (cancels in justnorm)

_COMPILED: dict = {}


class _Pools:
    def __init__(self, tc):
        self.tc = tc
        self._open = {}

    def open(self, name, **kw):
        cm = self.tc.tile_pool(name=name, **kw)
        pool = cm.__enter__()
        self._open[name] = cm
        return pool

    def close(self, *names):
        for name in names:
            cm = self._open.pop(name)
            cm.__exit__(None, None, None)

    def close_all(self):
        for name in reversed(list(self._open)):
            self.close(name)


def _declare_io(nc):
    io = {}
    io["htf"] = nc.dram_tensor("htf", [P, CCH, T], F8, kind="ExternalInput")
    io["htm"] = nc.dram_tensor("htm", [P, TCH, C], BF, kind="ExternalInput")
    io["wq"] = nc.dram_tensor("wq", [P, CCH, C], F8, kind="ExternalInput")
    io["wk"] = nc.dram_tensor("wk", [P, CCH, C], F8, kind="ExternalInput")
    io["wv"] = nc.dram_tensor("wv", [P, CCH, C], F8, kind="ExternalInput")
    io["wo"] = nc.dram_tensor("wo", [P, CCH, C], F8, kind="ExternalInput")
    io["wfc"] = nc.dram_tensor("wfc", [2, 4, P, 2, CCH, 512], F8, kind="ExternalInput")
    io["wpj"] = nc.dram_tensor("wpj", [P, KCH, C], F8, kind="ExternalInput")
    io["esc8"] = nc.dram_tensor("esc8", [P, H], F32, kind="ExternalInput")
    io["ind16"] = nc.dram_tensor("ind16", [P, CCH, H], BF, kind="ExternalInput")
    io["ident"] = nc.dram_tensor("ident", [P, P], BF, kind="ExternalInput")
    io["out"] = nc.dram_tensor("out", [P, TCH, C], BF, kind="ExternalOutput")
    return io


def _emit_preamble(nc, pl, io):
    """Constants + all state that crosses hardware-loop trip boundaries."""
    consts = pl.open("consts", bufs=1)
    st = {}
    st["ident"] = consts.tile([P, P], BF, name="ident")
    nc.sync.dma_start(out=st["ident"], in_=io["ident"].ap())
    st["esc8"] = consts.tile([P, H], F32, name="esc8")
    nc.sync.dma_start(out=st["esc8"], in_=io["esc8"].ap())
    st["ind16"] = consts.tile([P, CCH, H], BF, name="ind16")
    nc.sync.dma_start(out=st["ind16"], in_=io["ind16"].ap())
    st["ebias"] = consts.tile([P, 1], F32, name="ebias")
    nc.vector.memset(st["ebias"], EXP_BIAS)

    persist = pl.open("persist", bufs=1)
    # v in fp8, two half-zeroed copies: vz[s] has head-parity s features
    # live and the other parity zero, so AV DoubleRow can use M=128
    # stationaries that write both sub-heads' PSUM rows in one chain.
    st["vz"] = [persist.tile([P, TCH, C], F8, name=f"vz{s}") for s in range(2)]
    # feature-major q-hat, zero-padded per head (the other sub-head's 64
    # rows stay zero) so score matmuls run dense K=128 stationaries.
    st["qz"] = persist.tile([P, H, T], F8, name="qz")
    st["kfm"] = persist.tile([P, CCH, T], F8, name="kfm")
    st["esc_all"] = persist.tile([P, TCH, H], F32, name="esc_all")
    st["htm"] = persist.tile([P, TCH, C], BF, name="htm_sb")
    st["rs2i"] = persist.tile([P, TCH], F32, name="rs2i")
    for s in range(2):
        nc.vector.memset(st["vz"][s], 0)
    nc.vector.memset(st["qz"], 0)
    return st


def _weave(*unit_lists):
    """Emit thunks from several lists interleaved proportionally."""
    lists = [list(u) for u in unit_lists if u]
    idx = [0] * len(lists)
    total = sum(len(u) for u in lists)
    for _ in range(total):
        best = min(
            (i for i in range(len(lists)) if idx[i] < len(lists[i])),
            key=lambda i: idx[i] / len(lists[i]),
        )
        lists[best][idx[best]]()
        idx[best] += 1


class _Iter:
    """Emitters for one logical block iteration.  P-stage emitters write
    the persistent tiles (kfm/esc/qz/vz/htm/rs2i)."""

    def __init__(self, nc, io, st, pl, lr_a, lr_m):
        self.nc, self.io, self.st, self.pl = nc, io, st, pl
        self.lr_a, self.lr_m = lr_a, lr_m

    # ---------------- input DMAs ----------------
    def emit_input_dmas(self, htf, w_sbs, with_htm=True):
        nc, io, st = self.nc, self.io, self.st
        nc.scalar.dma_start(out=w_sbs["wo"], in_=io["wo"].ap())
        nc.sync.dma_start(out=htf, in_=io["htf"].ap())
        nc.sync.dma_start(out=w_sbs["wk"], in_=io["wk"].ap())
        nc.sync.dma_start(out=w_sbs["wq"], in_=io["wq"].ap())
        nc.scalar.dma_start(out=w_sbs["wv"], in_=io["wv"].ap())
        if with_htm:
            self.emit_htm_dma()

    def emit_htm_dma(self):
        nc, io, st = self.nc, self.io, self.st
        nc.scalar.dma_start(out=st["htm"], in_=io["htm"].ap())

    # ------------ h row norms: rs2i = ((1-la)/la)^2 / ||h||^2 ------------
    def hn_units(self, hnscr):
        nc, st = self.nc, self.st
        la = self.lr_a / (1.0 - self.lr_a)

        def unit(it):
            nscr = hnscr.tile([P, C], BF, name="nscr", tag="nscr")
            nc.scalar.activation(nscr, st["htm"][:, it, :], AF.Square,
                                 accum_out=st["rs2i"][:, it:it + 1])

        def tail():
            nc.vector.reciprocal(st["rs2i"], st["rs2i"])
            nc.vector.tensor_scalar_mul(st["rs2i"], st["rs2i"],
                                        1.0 / (la * la))

        return [lambda it=it: unit(it) for it in range(TCH)] + [tail]

    # ------------ K stage: k^T direct + per-head norms ------------
    def k_units(self, htf, wk, kps, rkps, ksqp):
        nc, st = self.nc, self.st
        rkp = rkps.tile([16, 2, 512], F32, name="rkp", tag="rkp")

        def unit(ci):
            kt = kps.tile([P, T], F32, name="kt", tag="kt")
            for cp in range(CCH // 2):
                lhs = wk[:, 2 * cp:2 * cp + 2, ci * P:(ci + 1) * P]
                for hf in range(2):
                    nc.tensor.matmul(
                        kt[:, hf * 512:(hf + 1) * 512], lhs,
                        htf[:, 2 * cp:2 * cp + 2, hf * 512:(hf + 1) * 512],
                        perf_mode=MM8,
                        start=(cp == 0), stop=(cp == CCH // 2 - 1),
                    )
            nc.vector.tensor_copy(st["kfm"][:, ci, :], kt)
            ksq = ksqp.tile([P, T], BF, name="ksq", tag="ksq")
            nc.scalar.activation(ksq, kt, AF.Square)
            for hf in range(2):
                nc.tensor.matmul(
                    rkp[:, hf], st["ind16"][:, ci, :],
                    ksq[:, hf * 512:(hf + 1) * 512],
                    start=(ci == 0), stop=(ci == CCH - 1),
                )

        self._rkp = rkp
        return [lambda ci=ci: unit(ci) for ci in range(CCH)]

    def esc_tail(self, escps, small):
        nc, st = self.nc, self.st
        rk_sb = small.tile([16, T], BF, name="rk_sb", tag="rk_sb")
        nc.vector.tensor_copy(rk_sb, self._rkp.rearrange("p a b -> p (a b)"))
        for tk in range(TCH):
            esct = escps.tile([P, H], BF, name="esct", tag="esct")
            nc.tensor.transpose(esct, rk_sb[:, tk * P:(tk + 1) * P],
                                st["ident"][0:16, 0:16])
            em = small.tile([P, H], F32, name="em", tag="em")
            nc.vector.reciprocal(em, esct)
            nc.scalar.sqrt(em, em)
            nc.vector.tensor_mul(st["esc_all"][:, tk, :], em, st["esc8"])

    # ------------ Q stage: token-major + normalize + transpose ------------
    def q_units(self, htf, wq, qhat, qps, tqps, qscr, small):
        nc, st = self.nc, self.st

        def q_tp(ci, g):
            tq = tqps.tile([P, 4, P], BF, name="tq", tag="tq")
            for jj in range(4):
                it_ = g * 4 + jj
                nc.tensor.transpose(
                    tq[:, jj], qhat[:, it_, ci * P:(ci + 1) * P], st["ident"]
                )
            tqv = tq.rearrange("p a b -> p (a b)")
            for sub in range(2):
                h = 2 * ci + sub
                nc.vector.tensor_copy(
                    st["qz"][sub * D:(sub + 1) * D, h, g * 512:(g + 1) * 512],
                    tqv[sub * D:(sub + 1) * D, :],
                )

        def unit(it):
            psq = qps.tile([P, 2, 512], F32, name="psq", tag="psq")
            for cp in range(CCH // 2):
                lhs = htf[:, 2 * cp:2 * cp + 2, it * P:(it + 1) * P]
                for hf in range(2):
                    nc.tensor.matmul(
                        psq[:, hf], lhs,
                        wq[:, 2 * cp:2 * cp + 2, hf * 512:(hf + 1) * 512],
                        perf_mode=MM8,
                        start=(cp == 0), stop=(cp == CCH // 2 - 1),
                    )
            psqv = psq.rearrange("p a b -> p (a b)")
            sqt = qscr.tile([P, C], BF, name="sqt", tag="sqt")
            nc.scalar.activation(sqt, psqv, AF.Square)
            rq = small.tile([P, H], F32, name="rq", tag="rq")
            nc.vector.reduce_sum(rq, sqt.rearrange("p (h d) -> p h d", h=H),
                                 axis=AX.X)
            nc.vector.reciprocal(rq, rq)
            nc.scalar.sqrt(rq, rq)
            nc.vector.tensor_mul(
                qhat[:, it, :].rearrange("p (h d) -> p h d", h=H),
                psq.rearrange("p a (g d) -> p (a g) d", d=D),
                rq.to_broadcast((P, H, D)),
            )

        units = []
        for it in range(TCH):
            units.append(lambda it=it: unit(it))
            if it == 3:
                units.extend(lambda ci=ci: q_tp(ci, 0) for ci in range(CCH))
        units.extend(lambda ci=ci: q_tp(ci, 1) for ci in range(CCH))
        return units

    # ------------ V stage: token-major -> vz interleaved fp8 ------------
    def v_units(self, htf, wv, vps):
        nc, st = self.nc, self.st

        def unit(it):
            psv = vps.tile([P, 2, 512], F32, name="psv", tag="psv")
            for cp in range(CCH // 2):
                lhs = htf[:, 2 * cp:2 * cp + 2, it * P:(it + 1) * P]
                for hf in range(2):
                    nc.tensor.matmul(
                        psv[:, hf], lhs,
                        wv[:, 2 * cp:2 * cp + 2, hf * 512:(hf + 1) * 512],
                        perf_mode=MM8,
                        start=(cp == 0), stop=(cp == CCH // 2 - 1),
                    )
            psv_v = psv.rearrange("p a (i d) -> p a i d", d=D)
            for s in range(2):
                nc.vector.tensor_copy(
                    st["vz"][s][:, it, :]
                    .rearrange("p (a i d) -> p a i d", a=2, d=D)[:, :, s::2, :],
                    psv_v[:, :, s::2, :],
                )

        return [lambda it=it: unit(it) for it in range(TCH)]

    # ---------------- Phase A: attention ----------------
    def emit_A(self, yfm, aps, app):
        nc, st = self.nc, self.st
        for hp in range(H // 2):
            ypsum = aps.tile([P, 2, 512], F32, name="ypsum", tag="ypsum",
                             bufs=1)
            p_sb = [
                app.tile([P, TCH, T], F8, name=f"p{sub}", tag=f"p{sub}")
                for sub in range(2)
            ]
            for tk in range(TCH):
                sps = []
                for sub in range(2):
                    h = hp * 2 + sub
                    sp = aps.tile([P, 2, 512], F32, name="sp", tag="sp",
                                  bufs=2)
                    for hf in range(2):
                        nc.tensor.matmul(
                            sp[:, hf],
                            st["kfm"][:, hp, tk * P:(tk + 1) * P],
                            st["qz"][:, h, hf * 512:(hf + 1) * 512],
                            start=True, stop=True,
                        )
                    sps.append(sp)
                for sub in range(2):
                    h = hp * 2 + sub
                    nc.scalar.activation(
                        out=p_sb[sub][:, tk, :],
                        in_=sps[sub].rearrange("p a b -> p (a b)"),
                        func=AF.Exp,
                        scale=st["esc_all"][:, tk, h:h + 1],
                        bias=st["ebias"],
                    )
                if tk % 2 == 1:
                    m = tk // 2
                    for sub in range(2):
                        for hf in range(2):
                            nc.tensor.matmul(
                                ypsum[:, hf],
                                st["vz"][sub][:, 2 * m:2 * m + 2,
                                              hp * P:(hp + 1) * P],
                                p_sb[sub][:, 2 * m:2 * m + 2,
                                          hf * 512:(hf + 1) * 512],
                                perf_mode=MM8,
                                start=(m == 0 and sub == 0),
                                stop=(m == TCH // 2 - 1 and sub == 1),
                            )
            nc.vector.tensor_scalar_mul(
                yfm[:, hp, :], ypsum.rearrange("p a b -> p (a b)"), YSCALE
            )

    # ------------ Phase O: out-proj + attention residual ------------
    def o_units(self, yfm, wo, h2a, opsp, oscr, small):
        nc, st = self.nc, self.st

        def unit(it):
            ops = opsp.tile([P, 2, 512], F32, name="ops", tag="ops")
            for cp in range(CCH // 2):
                lhs = yfm[:, 2 * cp:2 * cp + 2, it * P:(it + 1) * P]
                for hf in range(2):
                    nc.tensor.matmul(
                        ops[:, hf], lhs,
                        wo[:, 2 * cp:2 * cp + 2, hf * 512:(hf + 1) * 512],
                        perf_mode=MM8,
                        start=(cp == 0), stop=(cp == CCH // 2 - 1),
                    )
            opsv = ops.rearrange("p a b -> p (a b)")
            sb = small.tile([P, 1], F32, name="sb", tag="sb")
            nsq = oscr.tile([P, C], BF, name="nsq", tag="nsq")
            nc.scalar.activation(nsq, opsv, AF.Square, accum_out=sb)
            nc.scalar.activation(sb, sb, AF.Sqrt,
                                 scale=st["rs2i"][:, it:it + 1])
            nc.vector.reciprocal(sb, sb)
            acc = oscr.tile([P, C], F32, name="acc", tag="acc")
            nc.vector.scalar_tensor_tensor(
                out=acc, in0=opsv, scalar=sb, in1=st["htm"][:, it, :],
                op0=ALU.mult, op1=ALU.add,
            )
            s2 = small.tile([P, 1], F32, name="s2", tag="s2")
            nsq2 = oscr.tile([P, C], BF, name="nsq2", tag="nsq2")
            nc.scalar.activation(nsq2, acc, AF.Square, accum_out=s2)
            nc.scalar.sqrt(s2, s2)
            nc.vector.reciprocal(s2, s2)
            nc.vector.tensor_scalar_mul(h2a[:, it, :], acc, s2)

        return [lambda it=it: unit(it) for it in range(TCH)]

    # ------------ Phase T2: h2 -> feature-major fp8 ------------
    def t2_units(self, h2a, h2fm, tpps2):
        nc, st = self.nc, self.st

        def unit(ci, g):
            tp2 = tpps2.tile([P, 4, P], BF, name="tp2", tag="tp2")
            for jj in range(4):
                it = g * 4 + jj
                nc.tensor.transpose(
                    tp2[:, jj], h2a[:, it, ci * P:(ci + 1) * P], st["ident"]
                )
            nc.vector.tensor_scalar_mul(
                h2fm[:, ci, g * 512:(g + 1) * 512],
                tp2.rearrange("p a b -> p (a b)"), 8.0,
            )

        return [lambda ci=ci, g=g: unit(ci, g)
                for ci in range(CCH) for g in range(2)]

    # ------------ Phase F1: MLP up + SwiGLU ------------
    def emit_F1(self, h2fm, xm, f1w, f1ps, f1scr):
        nc, io = self.nc, self.io
        for jp in range(4):
            wu = f1w.tile([P, 2, CCH, 512], F8, name="wu", tag="wu")
            nc.sync.dma_start(out=wu, in_=io["wfc"].ap()[0, jp])
            wvt = f1w.tile([P, 2, CCH, 512], F8, name="wvt", tag="wvt")
            nc.scalar.dma_start(out=wvt, in_=io["wfc"].ap()[1, jp])
            for jj in range(2):
                for so in range(4):
                    oc = (jp * 2 + jj) * 4 + so
                    m0 = so * P
                    up = f1ps.tile([P, 2, 512], F32, name="up", tag="up")
                    vp = f1ps.tile([P, 2, 512], F32, name="vp", tag="vp")
                    for cp in range(CCH // 2):
                        for hf in range(2):
                            nc.tensor.matmul(
                                up[:, hf],
                                wu[:, jj, 2 * cp:2 * cp + 2, m0:m0 + P],
                                h2fm[:, 2 * cp:2 * cp + 2,
                                     hf * 512:(hf + 1) * 512],
                                perf_mode=MM8,
                                start=(cp == 0), stop=(cp == CCH // 2 - 1),
                            )
                    for cp in range(CCH // 2):
                        for hf in range(2):
                            nc.tensor.matmul(
                                vp[:, hf],
                                wvt[:, jj, 2 * cp:2 * cp + 2, m0:m0 + P],
                                h2fm[:, 2 * cp:2 * cp + 2,
                                     hf * 512:(hf + 1) * 512],
                                perf_mode=MM8,
                                start=(cp == 0), stop=(cp == CCH // 2 - 1),
                            )
                    sil = f1scr.tile([P, T], BF, name="sil", tag="sil")
                    nc.scalar.activation(
                        out=sil, in_=vp.rearrange("p a b -> p (a b)"),
                        func=AF.Silu, scale=1.0 / 8.0,
                    )
                    nc.vector.tensor_mul(
                        xm[:, oc, :], up.rearrange("p a b -> p (a b)"), sil
                    )

    # ------------ Phase F2: MLP down + MLP residual ------------
    def f2_units(self, xm, wpj, h2a, outsb, f2ps, f2scr, small):
        nc, io = self.nc, self.io
        lm = self.lr_m / (1.0 - self.lr_m)

        def unit(it):
            mp = f2ps.tile([P, 2, 512], F32, name="mp", tag="mp")
            for kp in range(KCH // 2):
                for hf in range(2):
                    nc.tensor.matmul(
                        mp[:, hf],
                        xm[:, 2 * kp:2 * kp + 2, it * P:(it + 1) * P],
                        wpj[:, 2 * kp:2 * kp + 2, hf * 512:(hf + 1) * 512],
                        perf_mode=MM8,
                        start=(kp == 0), stop=(kp == KCH // 2 - 1),
                    )
            mpv = mp.rearrange("p a b -> p (a b)")
            sb2 = small.tile([P, 1], F32, name="sb2", tag="sb2")
            nsq3 = f2scr.tile([P, C], BF, name="nsq3", tag="nsq3")
            nc.scalar.activation(nsq3, mpv, AF.Square, accum_out=sb2)
            nc.scalar.activation(sb2, sb2, AF.Sqrt, scale=1.0 / (lm * lm))
            nc.vector.reciprocal(sb2, sb2)
            acc2 = f2scr.tile([P, C], F32, name="acc2", tag="acc2")
            nc.vector.scalar_tensor_tensor(
                out=acc2, in0=mpv, scalar=sb2, in1=h2a[:, it, :],
                op0=ALU.mult, op1=ALU.add,
            )
            s3 = small.tile([P, 1], F32, name="s3", tag="s3")
            nsq4 = f2scr.tile([P, C], BF, name="nsq4", tag="nsq4")
            nc.scalar.activation(nsq4, acc2, AF.Square, accum_out=s3)
            nc.scalar.sqrt(s3, s3)
            nc.vector.reciprocal(s3, s3)
            nc.scalar.mul(outsb[:, it, :], acc2, s3)
            if it == TCH - 1:
                nc.scalar.dma_start(out=io["out"].ap(), in_=outsb)

        return [lambda it=it: unit(it) for it in range(TCH)]


def _emit_P_flat(nc, em, pl, pfx=""):
    """QKV/norm stages, standalone order (prologue and flat path).
    Returns w_sbs (wo still needed by the caller's O phase)."""
    htfp = pl.open(pfx + "htfp", bufs=1)
    wqkp = pl.open(pfx + "wqkp", bufs=1)
    qhatp = pl.open(pfx + "qhatp", bufs=1)
    psmall = pl.open(pfx + "psmall", bufs=4)
    htf = htfp.tile([P, CCH, T], F8, name="htf_sb")
    w_sbs = {
        "wk": wqkp.tile([P, CCH, C], F8, name="wk_sb"),
        "wq": wqkp.tile([P, CCH, C], F8, name="wq_sb"),
        "wv": wqkp.tile([P, CCH, C], F8, name="wv_sb"),
        "wo": wqkp.tile([P, CCH, C], F8, name="wo_sb"),
    }
    em.emit_input_dmas(htf, w_sbs)
    hnscr = pl.open(pfx + "hnscr", bufs=2, side="right")
    for u in em.hn_units(hnscr):
        u()
    pl.close(pfx + "hnscr")
    rkps = pl.open(pfx + "rk_psum", bufs=1, space="PSUM")
    kps = pl.open(pfx + "k_psum", bufs=2, space="PSUM")
    ksqp = pl.open(pfx + "ksqp", bufs=3, side="right")
    for u in em.k_units(htf, w_sbs["wk"], kps, rkps, ksqp):
        u()
    pl.close(pfx + "k_psum")
    escps = pl.open(pfx + "esc_psum", bufs=2, space="PSUM")
    em.esc_tail(escps, psmall)
    pl.close(pfx + "esc_psum", pfx + "ksqp", pfx + "rk_psum")
    qhat = qhatp.tile([P, TCH, C], BF, name="qhat")
    qps = pl.open(pfx + "q_psum", bufs=2, space="PSUM")
    tqps = pl.open(pfx + "tq_psum", bufs=2, space="PSUM")
    qscr = pl.open(pfx + "qscr", bufs=2)
    for u in em.q_units(htf, w_sbs["wq"], qhat, qps, tqps, qscr, psmall):
        u()
    pl.close(pfx + "qscr", pfx + "tq_psum", pfx + "q_psum")
    vps = pl.open(pfx + "v_psum", bufs=2, space="PSUM")
    for u in em.v_units(htf, w_sbs["wv"], vps):
        u()
    pl.close(pfx + "v_psum")
    return w_sbs


def _emit_flat(nc, tc, io, st, pl, lr_a, lr_m, stop_after="full"):
    """One self-contained iteration: P, A, O, T2, F1, F2 (graded path)."""
    em = _Iter(nc, io, st, pl, lr_a, lr_m)

    def _dump_and_stop(srcs):
        dmp = pl.open("dmp", bufs=2)
        for i, s in enumerate(srcs[:TCH]):
            dt_ = dmp.tile([P, C], BF, name=f"dt{i}", tag="dt")
            nc.vector.tensor_copy(dt_, s)
            nc.sync.dma_start(out=io["out"].ap()[:, i, :], in_=dt_)
        pl.close("dmp")
        for name in [n for n in reversed(list(pl._open))
                     if n not in ("consts", "persist")]:
            pl.close(name)

    h2ap = pl.open("h2ap", bufs=1)
    h2a = h2ap.tile([P, TCH, C], BF, name="h2a")
    small = pl.open("small", bufs=4)
    w_sbs = _emit_P_flat(nc, em, pl)
    if stop_after == "p":
        _dump_and_stop([st["kfm"][:, i, :] for i in range(TCH)])
        return

    yfmp = pl.open("yfmp", bufs=1, side="right")
    yfm = yfmp.tile([P, CCH, T], F8, name="yfm")
    aps = pl.open("att_psum", bufs=1, space="PSUM")
    app = pl.open("att_p", bufs=2)
    em.emit_A(yfm, aps, app)
    pl.close("att_p", "att_psum")
    if stop_after == "a":
        _dump_and_stop([yfm[:, i, :] for i in range(TCH)])
        return

    opsp = pl.open("o_psum", bufs=3, space="PSUM")
    oscr = pl.open("o_scr", bufs=3)
    for u in em.o_units(yfm, w_sbs["wo"], h2a, opsp, oscr, small):
        u()
    pl.close("o_scr", "o_psum", "yfmp")
    if stop_after == "o":
        _dump_and_stop([h2a[:, i, :] for i in range(TCH)])
        return
    pl.close("psmall", "qhatp", "wqkp", "htfp")

    h2fmp = pl.open("h2fmp", bufs=1, side="right")
    h2fm = h2fmp.tile([P, CCH, T], F8, name="h2fm")
    tpps2 = pl.open("tp2_psum", bufs=3, space="PSUM")
    for u in em.t2_units(h2a, h2fm, tpps2):
        u()
    pl.close("tp2_psum")

    xmp = pl.open("xmp", bufs=1)
    xm = xmp.tile([P, KCH, T], F8, name="xm")
    wpjp = pl.open("wpjp", bufs=1)
    wpj = wpjp.tile([P, KCH, C], F8, name="wpj")
    for q in range(2):
        nc.scalar.dma_start(out=wpj[:, q * 16:(q + 1) * 16, :],
                            in_=io["wpj"].ap()[:, q * 16:(q + 1) * 16, :])
    f1w = pl.open("f1w", bufs=2, side="right")
    f1ps = pl.open("f1_psum", bufs=2, space="PSUM")
    f1scr = pl.open("f1scr", bufs=2, side="right")
    em.emit_F1(h2fm, xm, f1w, f1ps, f1scr)
    pl.close("f1scr", "f1w", "h2fmp", "f1_psum")

    f2ps = pl.open("f2_psum", bufs=3, space="PSUM")
    f2scr = pl.open("f2scr", bufs=3)
    outp = pl.open("outp", bufs=1)
    outsb = outp.tile([P, TCH, C], BF, name="outsb")
    for u in em.f2_units(xm, wpj, h2a, outsb, f2ps, f2scr, small):
        u()
    pl.close("outp", "f2scr", "f2_psum", "wpjp", "xmp")
    pl.close("small", "h2ap")


def _emit_pipe_body(nc, tc, io, st, pl, lr_a, lr_m):
    """Pipelined loop body: A(i) + MLP(i), with P(i+1) stages woven into
    the MLP sections."""
    em = _Iter(nc, io, st, pl, lr_a, lr_m)

    htfp = pl.open("htfp", bufs=1)
    h2ap = pl.open("h2ap", bufs=1)
    small = pl.open("small", bufs=4)
    wvp = pl.open("wvp", bufs=1)
    wqp = pl.open("wqp", bufs=1)
    wkop = pl.open("wkop", bufs=1)
    htf = htfp.tile([P, CCH, T], F8, name="htf_sb")
    w_sbs = {
        "wv": wvp.tile([P, CCH, C], F8, name="wv_sb"),
        "wq": wqp.tile([P, CCH, C], F8, name="wq_sb"),
        "wk": wkop.tile([P, CCH, C], F8, name="wk_sb"),
        "wo": wkop.tile([P, CCH, C], F8, name="wo_sb"),
    }
    h2a = h2ap.tile([P, TCH, C], BF, name="h2a")
    # inputs for i+1 (WAR semaphores gate each against its last reader);
    # htm(i+1) must wait until after O(i)'s htm(i) reads, so it is
    # emitted at the T2~Q section instead.
    em.emit_input_dmas(htf, w_sbs, with_htm=False)

    # ---- A(i) ----
    yfmp = pl.open("yfmp", bufs=1, side="right")
    yfm = yfmp.tile([P, CCH, T], F8, name="yfm")
    aps = pl.open("att_psum", bufs=1, space="PSUM")
    app = pl.open("att_p", bufs=2)
    em.emit_A(yfm, aps, app)
    pl.close("att_p", "att_psum")

    # ---- O(i) ~ K(i+1) ----
    rkps = pl.open("rk_psum", bufs=1, space="PSUM")
    opsp = pl.open("o_psum", bufs=2, space="PSUM")
    oscr = pl.open("o_scr", bufs=3)
    kps = pl.open("k_psum", bufs=1, space="PSUM")
    ksqp = pl.open("ksqp", bufs=3, side="right")
    _weave(em.o_units(yfm, w_sbs["wo"], h2a, opsp, oscr, small),
           em.k_units(htf, w_sbs["wk"], kps, rkps, ksqp))
    pl.close("k_psum", "o_scr", "o_psum")
    escps = pl.open("esc_psum", bufs=2, space="PSUM")
    em.esc_tail(escps, small)
    pl.close("esc_psum", "ksqp", "rk_psum", "yfmp", "wkop")

    # ---- T2(i) ~ Q(i+1) ----
    em.emit_htm_dma()
    qhatp = pl.open("qhatp", bufs=1)
    qhat = qhatp.tile([P, TCH, C], BF, name="qhat")
    h2fmp = pl.open("h2fmp", bufs=1, side="right")
    h2fm = h2fmp.tile([P, CCH, T], F8, name="h2fm")
    tpps2 = pl.open("tp2_psum", bufs=2, space="PSUM")
    qps = pl.open("q_psum", bufs=2, space="PSUM")
    tqps = pl.open("tq_psum", bufs=2, space="PSUM")
    qscr = pl.open("qscr", bufs=2)
    _weave(em.t2_units(h2a, h2fm, tpps2),
           em.q_units(htf, w_sbs["wq"], qhat, qps, tqps, qscr, small))
    pl.close("qscr", "tq_psum", "q_psum", "tp2_psum", "qhatp", "wqp")

    # ---- F1(i) ----
    xmp = pl.open("xmp", bufs=1)
    xm = xmp.tile([P, KCH, T], F8, name="xm")
    wpjp = pl.open("wpjp", bufs=1)
    wpj = wpjp.tile([P, KCH, C], F8, name="wpj")
    for q in range(2):
        nc.scalar.dma_start(out=wpj[:, q * 16:(q + 1) * 16, :],
                            in_=io["wpj"].ap()[:, q * 16:(q + 1) * 16, :])
    f1w = pl.open("f1w", bufs=2, side="right")
    f1ps = pl.open("f1_psum", bufs=2, space="PSUM")
    f1scr = pl.open("f1scr", bufs=2, side="right")
    em.emit_F1(h2fm, xm, f1w, f1ps, f1scr)
    pl.close("f1scr", "f1w", "h2fmp", "f1_psum")

    # ---- F2(i) ~ V(i+1) ~ hn(i+1) ----
    f2ps = pl.open("f2_psum", bufs=2, space="PSUM")
    f2scr = pl.open("f2scr", bufs=3)
    outp = pl.open("outp", bufs=1)
    outsb = outp.tile([P, TCH, C], BF, name="outsb")
    vps = pl.open("v_psum", bufs=2, space="PSUM")
    hnscr = pl.open("hnscr", bufs=2, side="right")
    _weave(em.f2_units(xm, wpj, h2a, outsb, f2ps, f2scr, small),
           em.v_units(htf, w_sbs["wv"], vps),
           em.hn_units(hnscr))
    pl.close("hnscr", "v_psum", "outp", "f2scr", "f2_psum", "wpjp", "xmp")
    pl.close("wvp", "small", "h2ap", "htfp")


def build_program(lr_a: float, lr_m: float, reps: int = 1, loop: int = 0,
                  stop_after: str = "full"):
    key = (round(lr_a, 12), round(lr_m, 12), reps, loop, stop_after)
    if key in _COMPILED:
        return _COMPILED[key]
    nc = bacc.Bacc("TRN2", target_bir_lowering=False, debug=False, num_devices=NCORES)
    with tile.TileContext(nc) as tc:
        io = _declare_io(nc)
        pl = _Pools(tc)
        st = _emit_preamble(nc, pl, io)
        if loop:
            if stop_after == "full":
                em0 = _Iter(nc, io, st, pl, lr_a, lr_m)
                _emit_P_flat(nc, em0, pl, pfx="pro_")
                pl.close("pro_psmall", "pro_qhatp", "pro_wqkp", "pro_htfp")
                with tc.For_i(0, loop, 1):
                    _emit_pipe_body(nc, tc, io, st, pl, lr_a, lr_m)
            else:
                with tc.For_i(0, loop, 1):
                    _emit_flat(nc, tc, io, st, pl, lr_a, lr_m, stop_after)
        else:
            for _ in range(reps):
                _emit_flat(nc, tc, io, st, pl, lr_a, lr_m, stop_after)
        pl.close_all()
    nc.compile()
    _COMPILED[key] = nc
    return nc


def prep_inputs(h, Wq, Wk, Wv, Wo, Wfc, Wproj, sqk, suv, attn_alpha, mlp_alpha):
    """Host-side sharding/layout. Returns (in_maps list per core, lr_a, lr_m)."""
    h = np.asarray(h, np.float32)
    Wq, Wk, Wv, Wo = (np.asarray(w, np.float64) for w in (Wq, Wk, Wv, Wo))
    Wfc = np.asarray(Wfc, np.float32)
    Wproj = np.asarray(Wproj, np.float32)
    sqk = np.asarray(sqk, np.float64)
    suv = np.asarray(suv, np.float64)
    attn_alpha = np.asarray(attn_alpha, np.float64)
    mlp_alpha = np.asarray(mlp_alpha, np.float64)

    sqk_s = sqk * (SQK_INIT / BASE_SCALE)
    s_h = sqk_s.reshape(H, D)
    assert np.allclose(s_h, s_h[:, :1]), "sqk must be constant per head"
    s2 = (s_h[:, 0] ** 2) * np.sqrt(D)
    esc8 = np.ascontiguousarray(np.broadcast_to(s2.astype(np.float32), (P, H)))

    lr_a_v = np.abs(attn_alpha * (ATTN_ALPHA_INIT / BASE_SCALE))
    lr_m_v = np.abs(mlp_alpha * (MLP_ALPHA_INIT / BASE_SCALE))
    assert np.allclose(lr_a_v, lr_a_v[0]) and np.allclose(lr_m_v, lr_m_v[0]), \
        "alpha must be constant"
    lr_a = float(lr_a_v[0])
    lr_m = float(lr_m_v[0])

    def wt_tiles_norm_f8(W):  # [out, in] -> [128, CCH, out] fp8, unit cols x16
        Wn = W / np.linalg.norm(W, axis=0, keepdims=True)
        Wn = Wn * WSCALE
        return np.ascontiguousarray(
            Wn.T.reshape(CCH, P, W.shape[0]).transpose(1, 0, 2)
        ).astype(FP8)

    wq_t, wk_t, wv_t, wo_t = (wt_tiles_norm_f8(w) for w in (Wq, Wk, Wv, Wo))

    suv_s = suv * (SUV_INIT / 1.0 * np.sqrt(C))
    wfc_f = (Wfc.astype(np.float64) * suv_s[:, None]).astype(np.float32)  # [F, C]
    # [F, C] -> [kind(2), jpair(4), P, j-in-pair(2), CCH, 512]; per-partition
    # runs of 8 KB keep the HWDGE descriptor count at 128 per 1 MB load.
    wfc_j = np.clip(
        wfc_f.T.reshape(CCH, P, 16, 512).transpose(2, 1, 0, 3), -224, 224
    )  # [16(j), P, CCH, 512]
    wfc_t = np.ascontiguousarray(
        wfc_j.reshape(2, 4, 2, P, CCH, 512).transpose(0, 1, 3, 2, 4, 5)
    ).astype(FP8)
    wpj_t = np.ascontiguousarray(
        np.clip(Wproj.T.reshape(KCH, P, C).transpose(1, 0, 2) * 64.0, -224, 224)
    ).astype(FP8)

    # indicator stationaries for the per-head ||k||^2 partition reduction:
    # variant ci maps partitions [0,64) -> head 2ci, [64,128) -> head 2ci+1
    ind16 = np.zeros((P, CCH, H), dtype=BF16)
    for ci in range(CCH):
        ind16[0:D, ci, 2 * ci] = 1.0
        ind16[D:P, ci, 2 * ci + 1] = 1.0

    shared = {
        "wq": wq_t, "wk": wk_t, "wv": wv_t, "wo": wo_t,
        "wfc": wfc_t, "wpj": wpj_t, "esc8": esc8, "ind16": ind16,
        "ident": np.eye(P, dtype=np.float32).astype(BF16),
    }
    in_maps = []
    for b in range(NCORES):
        htf = np.ascontiguousarray(
            h[b].T.reshape(CCH, P, T).transpose(1, 0, 2)
        ).astype(FP8)
        htm = np.ascontiguousarray(
            h[b].reshape(TCH, P, C).transpose(1, 0, 2)).astype(BF16)
        in_maps.append({"htf": htf, "htm": htm, **shared})
    return in_maps, lr_a, lr_m


def kernel(h, Wq, Wk, Wv, Wo, Wfc, Wproj, sqk, suv, attn_alpha, mlp_alpha):
    in_maps, lr_a, lr_m = prep_inputs(
        h, Wq, Wk, Wv, Wo, Wfc, Wproj, sqk, suv, attn_alpha, mlp_alpha
    )
    nc = build_program(lr_a, lr_m)
    from concourse.bass_utils import run_bass_kernel_spmd

    res = run_bass_kernel_spmd(nc, in_maps, core_ids=list(range(NCORES)))
    out = np.stack(
        [res.results[b]["out"].astype(np.float32).transpose(1, 0, 2)
         .reshape(T, C)
         for b in range(NCORES)], axis=0
    )
    return out
